# revision 9
# baseline (speedup 1.0000x reference)
"""Trainium2 Bass kernel for RandomSparseNewMlp.

Math (reference):
    attn = (einsum('ds,td->st', fc1_w, fc2_w) + fc2_b) * sparse_mask   # [1024, 1024]
    out  = gelu_erf(einsum('bds,st->bdt', x, attn))                    # [64, 768, 1024]

Strategy (8 cores, SPMD, two NEFF dispatches, no collectives):
  NEFF A ("attn"): tensor-parallel shard of the hidden dim d=4096: core c
    contracts its 512-row K-slice of fc1/fc2^T into a full [1024, 1024]
    fp16 partial product (pure matmul, no bias/mask on device).  The host
    sums the 8 partials, adds the bias and applies the sparse mask
    (elementwise glue, like the gather/unshard step).  This loads only
    2.1 MB of weights per core vs 6.5 MB for a 2D-sharded attn, so the
    NEFF is no longer DMA-gated.
  NEFF B ("mlp"): data-parallel shard of x over batch; core c handles
    rows [c*6144, (c+1)*6144) of the flattened [49152, 1024] x, computes
    gelu(x @ attn) with the gathered attn as a replicated input.

Latency tricks (from NTFF traces):
  * The PE clock is HAM-gated: 1.2 GHz until ~3.4 us of sustained matmul
    activity, then 2.4 GHz.  Both NEFFs open with a short burst of dummy
    matmuls on a zeroed SBUF tile so the gate opens while the first DMAs
    are still in flight.
  * DMA_DIRECT2D descriptor builds cost ~0.65 us *per issue* on the
    issuing engine, so issues are split between the two HWDGE engines
    (Sync + Scalar) and ordered so the k-chunks land just ahead of the
    matmuls that consume them.
  * NEFF B starts with a k-major prologue over the first 4 row-blocks
    (all 8 PSUM banks) so real matmuls start as soon as attn chunk 0
    lands (~8.6 us) instead of waiting for the full 2 MB attn transfer.

  All matmul operands are fp16: full PE rate (1 row/cycle), ~5e-4
  element precision, half the HBM traffic of fp32.  PSUM accumulation
  is fp32; outputs are evicted as fp16 (host upcasts) to halve the
  output DMA.  x is host-pre-transposed (xT layout [1024, rows]) so the
  contraction dim lands on SBUF partitions with clean contiguous DMA.
  GELU (erf-exact) is fused into the PSUM->SBUF eviction on ScalarE.
"""

import numpy as np
from contextlib import ExitStack

import concourse.bass as bass  # noqa: F401  (engine registration side effects)
import concourse.mybir as mybir
import concourse.tile as tile
from concourse import bacc
from concourse import bass_utils

P = 128
B, D = 64, 768
IN_F, HID_F, OUT_F = 1024, 4096, 1024
N_CORES = 8
ROWS = B * D                    # 49152
ROWS_PC = ROWS // N_CORES       # 6144
S_TILES = IN_F // P             # 8
K_CHUNKS = IN_F // P            # 8
RT = ROWS_PC // P               # 48
NB = 512                        # matmul moving free dim / PSUM bank
KPC = HID_F // N_CORES          # 512 contraction rows per core (NEFF A)
KC = KPC // P                   # 4 k-chunks per core (NEFF A)
PRO = 4                         # NEFF B prologue row-blocks (uses all 8 banks)
WARM_A = 30                     # HAM warmup matmuls (~3.2 us at 1.2 GHz)
WARM_B = 30                     # bridge to first DMA completion (~11 us)

F32 = mybir.dt.float32
F16 = mybir.dt.float16


def _warmup(nc, pool, ppool, psum_name, n):
    """Dummy matmuls to open the PE HAM clock gate during the DMA fill.

    The psum scratch reuses the main pool's tag (same name) so it cycles
    through the same 8 bank slots instead of claiming its own tag group.
    """
    wz = pool.tile([P, P], F16, name="warm")
    nc.vector.memset(wz, 0.0)
    wp = ppool.tile([P, NB], F32, name=psum_name)
    for _ in range(n):
        nc.tensor.matmul(wp[:, 0:P], wz, wz, start=True, stop=True)


def _trace_attn_kernel(tc, partial, fc1c, fc2tc):
    """partial[1024,1024] (fp16) = fc1c^T @ fc2tc for this core's K-slice.

    fc1c  [512, 1024] fp16 : fc1 rows for this core's K-slice (s columns)
    fc2tc [512, 1024] fp16 : fc2^T rows for the same K-slice (t columns)
    """
    nc = tc.nc
    fc1_r = fc1c.rearrange("(k p) s -> p k s", p=P)      # [128, 4, 1024]
    fc2_r = fc2tc.rearrange("(k p) t -> p k t", p=P)     # [128, 4, 1024]
    # partial is [th, p, sb, t]: each DRAM partition row is 8 KB
    # contiguous, so the output DMA uses 8 KB descriptors instead of the
    # 1 KB ones a [s, t] layout would force (4x fewer, ~4x faster drain).
    out_r = partial.rearrange("(th p) (sb t) -> th p sb t", p=P, sb=8)

    with ExitStack() as ctx:
        spool = ctx.enter_context(tc.tile_pool(name="spool", bufs=1))
        opool = ctx.enter_context(tc.tile_pool(name="opool", bufs=1))
        ppool = ctx.enter_context(tc.tile_pool(name="ppool", bufs=8, space="PSUM"))
        _warmup(nc, spool, ppool, 'ap', WARM_A)
        f1_sb = spool.tile([P, KC, IN_F], F16)
        f2_sb = spool.tile([P, KC, OUT_F], F16)
        # Paired per-chunk DMAs on both HWDGE engines so chunk k lands
        # ~1.4 us after chunk k-1 while matmuls consume one per 1.7 us.
        # Only the t-half wave 0 needs of fc2 goes out up front; the
        # second halves follow once the critical stream is in flight.
        for k in range(KC):
            nc.sync.dma_start(f1_sb[:, k, :], fc1_r[:, k, :])
            nc.scalar.dma_start(f2_sb[:, k, 0:NB], fc2_r[:, k, 0:NB])
        for k in range(KC):
            nc.scalar.dma_start(f2_sb[:, k, NB:OUT_F], fc2_r[:, k, NB:OUT_F])
        # Wave 0 (t-half 0): k outer / s inner so each freshly arrived
        # chunk pair feeds all 8 stationary loads before the next chunk
        # is needed.  Evictions on VectorE, output via Sync.
        psums = [ppool.tile([P, NB], F32, name="ap") for _ in range(8)]
        for k in range(KC):
            for sb in range(8):
                nc.tensor.matmul(
                    psums[sb],
                    f1_sb[:, k, sb * P:(sb + 1) * P],
                    f2_sb[:, k, 0:NB],
                    start=(k == 0),
                    stop=(k == KC - 1),
                )
        ot0 = opool.tile([P, 8, NB], F16, name="ot0")
        for sb in range(8):
            # Evictions round-robin Vector/Scalar (~0.68 us each, and
            # GpSimd has no PSUM port) so the out DMA isn't gated on a
            # single engine draining all eight banks.
            if sb % 2 == 0:
                nc.vector.tensor_copy(ot0[:, sb, :], psums[sb])
            else:
                nc.scalar.copy(ot0[:, sb, :], psums[sb])
        nc.sync.dma_start(out_r[0], ot0)
        # Wave 1 (t-half 1): all data resident -> s outer / k inner so
        # each s-block completes early and is evicted while the next
        # computes; output in two halves via Scalar (idle by now).
        ot1 = opool.tile([P, 8, NB], F16, name="ot1")
        for sb in range(8):
            p = ppool.tile([P, NB], F32, name="ap")
            for k in range(KC):
                nc.tensor.matmul(
                    p,
                    f1_sb[:, k, sb * P:(sb + 1) * P],
                    f2_sb[:, k, NB:OUT_F],
                    start=(k == 0),
                    stop=(k == KC - 1),
                )
            if sb % 2 == 0:
                nc.vector.tensor_copy(ot1[:, sb, :], p)
            else:
                nc.scalar.copy(ot1[:, sb, :], p)
            if sb == 3:
                nc.scalar.dma_start(out_r[1, :, 0:4, :], ot1[:, 0:4, :])
        nc.sync.dma_start(out_r[1, :, 4:8, :], ot1[:, 4:8, :])


def _trace_mlp_kernel(tc, out, attn, xt):
    """out[6144,1024] (fp16) = gelu(xT^T @ attn) for this core's row shard."""
    nc = tc.nc
    gelu = mybir.ActivationFunctionType.Gelu
    attn_r = attn.rearrange("(k p) t -> p k t", p=P)    # [128, 8, 1024]

    with ExitStack() as ctx:
        consts = ctx.enter_context(tc.tile_pool(name="consts", bufs=1))
        xpool = ctx.enter_context(tc.tile_pool(name="xpool", bufs=8))
        opool = ctx.enter_context(tc.tile_pool(name="opool", bufs=3))
        mpool = ctx.enter_context(tc.tile_pool(name="main_psum", bufs=8, space="PSUM"))
        _warmup(nc, consts, mpool, 'mp', WARM_B)
        attn_sb = consts.tile([P, S_TILES, OUT_F], F16)

        # Interleaved per-chunk attn / x-strip issues on Sync, ordered so
        # attn chunk k lands just before the prologue's k-th sweep.
        xs_t = []

        def xs_dma(rt):
            # xt is host-pre-shuffled to [rt, p, k, r]: each strip is 128
            # partition rows x 2 KB contiguous (128 DMA descriptors, not
            # 1024 256-byte ones).
            xs = xpool.tile([P, K_CHUNKS, P], F16, name="xs")
            nc.sync.dma_start(xs, xt[rt * P:(rt + 1) * P, :])
            return xs

        # attn chunks stream per-k on Sync; the prologue's x strips go
        # out in parallel on Scalar (idle until the first activations).
        for k in range(K_CHUNKS):
            nc.sync.dma_start(attn_sb[:, k:k + 1, :], attn_r[:, k:k + 1, :])
        for rt in range(PRO):
            xs = xpool.tile([P, K_CHUNKS, P], F16, name="xs")
            eng = nc.scalar if rt < 2 else nc.gpsimd
            eng.dma_start(xs, xt[rt * P:(rt + 1) * P, :])
            xs_t.append(xs)

        # Prologue: k-major over row-blocks 0..3 (8 PSUM banks) — matmuls
        # start on attn chunk 0 instead of the full attn transfer.
        pro_ps = []
        for rt in range(PRO):
            pro_ps.append((mpool.tile([P, NB], F32, name="mp"),
                           mpool.tile([P, NB], F32, name="mp")))
        for k in range(K_CHUNKS):
            for rt in range(PRO):
                nc.tensor.matmul(
                    pro_ps[rt][0], xs_t[rt][:, k, :], attn_sb[:, k, 0:NB],
                    start=(k == 0), stop=(k == K_CHUNKS - 1),
                )
                nc.tensor.matmul(
                    pro_ps[rt][1], xs_t[rt][:, k, :], attn_sb[:, k, NB:OUT_F],
                    start=(k == 0), stop=(k == K_CHUNKS - 1),
                )
        for rt in range(PRO):
            ot = opool.tile([P, OUT_F], F16, name="ot")
            nc.scalar.activation(ot[:, 0:NB], pro_ps[rt][0], gelu)
            nc.scalar.activation(ot[:, NB:OUT_F], pro_ps[rt][1], gelu)
            nc.scalar.dma_start(out[rt * P:(rt + 1) * P, :], ot)

        # Steady state: row-block major; x strips prefetched 8 deep via
        # the pool; GELU eviction + output DMA issue both on ScalarE.
        for rt in range(PRO, RT):
            xs = xs_dma(rt)
            pa = mpool.tile([P, NB], F32, name="mp")
            pb = mpool.tile([P, NB], F32, name="mp")
            for k in range(K_CHUNKS):
                nc.tensor.matmul(
                    pa, xs[:, k, :], attn_sb[:, k, 0:NB],
                    start=(k == 0), stop=(k == K_CHUNKS - 1),
                )
            for k in range(K_CHUNKS):
                nc.tensor.matmul(
                    pb, xs[:, k, :], attn_sb[:, k, NB:OUT_F],
                    start=(k == 0), stop=(k == K_CHUNKS - 1),
                )
            ot = opool.tile([P, OUT_F], F16, name="ot")
            nc.scalar.activation(ot[:, 0:NB], pa, gelu)
            nc.scalar.activation(ot[:, NB:OUT_F], pb, gelu)
            nc.scalar.dma_start(out[rt * P:(rt + 1) * P, :], ot)


_NC_CACHE = {}
LAST_RESULTS = None


def _build_attn():
    if "attn" in _NC_CACHE:
        return _NC_CACHE["attn"]
    nc = bacc.Bacc("TRN2", target_bir_lowering=False, debug=False,
                   num_devices=N_CORES)
    fc1c = nc.dram_tensor("fc1c", [KPC, IN_F], F16, kind="ExternalInput").ap()
    fc2tc = nc.dram_tensor("fc2tc", [KPC, OUT_F], F16, kind="ExternalInput").ap()
    partial = nc.dram_tensor("partial", [2 * P, 8 * NB], F16,
                             kind="ExternalOutput").ap()
    with tile.TileContext(nc) as tc:
        _trace_attn_kernel(tc, partial, fc1c, fc2tc)
    nc.compile()
    _NC_CACHE["attn"] = nc
    return nc


def _build_mlp():
    if "mlp" in _NC_CACHE:
        return _NC_CACHE["mlp"]
    nc = bacc.Bacc("TRN2", target_bir_lowering=False, debug=False,
                   num_devices=N_CORES)
    attn = nc.dram_tensor("attn", [IN_F, OUT_F], F16, kind="ExternalInput").ap()
    xt = nc.dram_tensor("xt", [ROWS_PC, IN_F], F16, kind="ExternalInput").ap()
    out = nc.dram_tensor("out", [ROWS_PC, OUT_F], F16, kind="ExternalOutput").ap()
    with tile.TileContext(nc) as tc:
        _trace_mlp_kernel(tc, out, attn, xt)
    nc.compile()
    _NC_CACHE["mlp"] = nc
    return nc


def _run(nc, in_maps, **kwargs):
    return bass_utils.run_bass_kernel_spmd(
        nc, in_maps, core_ids=list(range(N_CORES)), **kwargs
    )


def kernel(x, fc1_w, fc2_w, fc2_b, sparse_mask, **run_kwargs):
    global LAST_RESULTS
    nc_a = _build_attn()
    nc_b = _build_mlp()

    # --- host prep: fp16 K-slices of the weights (layout only) ---
    fc1_16 = np.asarray(fc1_w, np.float32).astype(np.float16)      # [4096, 1024]
    fc2t_16 = np.asarray(fc2_w, np.float32).T.astype(np.float16)   # [4096, 1024]

    in_maps_a = []
    for c in range(N_CORES):
        sl = slice(c * KPC, (c + 1) * KPC)
        in_maps_a.append({
            "fc1c": np.ascontiguousarray(fc1_16[sl]),
            "fc2tc": np.ascontiguousarray(fc2t_16[sl]),
        })

    res_a = _run(nc_a, in_maps_a, **run_kwargs)

    # --- host: sum K-partials, add bias, apply mask (elementwise glue) ---
    acc = np.zeros((2, P, 8, NB), np.float32)
    for c in range(N_CORES):
        acc += res_a.results[c]["partial"].reshape(2, P, 8, NB)
    # [th, p, sb, t] -> [sb*128+p, th*512+t]
    acc = acc.transpose(2, 1, 0, 3).reshape(IN_F, OUT_F)
    attn_full = ((acc + np.asarray(fc2_b, np.float32))
                 * np.asarray(sparse_mask, np.float32)).astype(np.float16)

    x_flat = np.asarray(x, np.float32).reshape(ROWS, IN_F)
    in_maps_b = []
    for c in range(N_CORES):
        # [rt, r, k, p] -> [rt, p, k, r]: strip rt*128+p holds the 8
        # stationary k-blocks for x row-block rt contiguously.
        xs_c = np.ascontiguousarray(
            x_flat[c * ROWS_PC:(c + 1) * ROWS_PC]
            .reshape(RT, P, K_CHUNKS, P)
            .transpose(0, 3, 2, 1)
            .reshape(ROWS_PC, IN_F)
            .astype(np.float16)
        )
        in_maps_b.append({"attn": attn_full, "xt": xs_c})

    res_b = _run(nc_b, in_maps_b, **run_kwargs)
    LAST_RESULTS = (res_a, res_b)
    out = np.concatenate(
        [res_b.results[c]["out"] for c in range(N_CORES)], axis=0
    ).astype(np.float32)
    return out.reshape(B, D, OUT_F)


# revision 10
# speedup vs baseline: 1.1723x; 1.1723x over previous
"""Trainium2 Bass kernel for RandomSparseNewMlp.

Math (reference):
    attn = (einsum('ds,td->st', fc1_w, fc2_w) + fc2_b) * sparse_mask   # [1024, 1024]
    out  = gelu_erf(einsum('bds,st->bdt', x, attn))                    # [64, 768, 1024]

Strategy (8 cores, SPMD, two NEFF dispatches, no collectives):
  NEFF A ("attn"): tensor-parallel shard of the hidden dim d=4096: core c
    contracts its 512-row K-slice of fc1/fc2^T into a full [1024, 1024]
    fp16 partial product (pure matmul, no bias/mask on device).  The host
    sums the 8 partials, adds the bias and applies the sparse mask
    (elementwise glue, like the gather/unshard step).  This loads only
    2.1 MB of weights per core vs 6.5 MB for a 2D-sharded attn, so the
    NEFF is no longer DMA-gated.
  NEFF B ("mlp"): data-parallel shard of x over batch; core c handles
    rows [c*6144, (c+1)*6144) of the flattened [49152, 1024] x, computes
    gelu(x @ attn) with the gathered attn as a replicated input.

Latency tricks (from NTFF traces):
  * The PE clock is HAM-gated: 1.2 GHz until ~3.4 us of sustained matmul
    activity, then 2.4 GHz.  Both NEFFs open with a short burst of dummy
    matmuls on a zeroed SBUF tile so the gate opens while the first DMAs
    are still in flight.
  * DMA_DIRECT2D descriptor builds cost ~0.65 us *per issue* on the
    issuing engine, so issues are split between the two HWDGE engines
    (Sync + Scalar) and ordered so the k-chunks land just ahead of the
    matmuls that consume them.
  * NEFF B starts with a k-major prologue over the first 4 row-blocks
    (all 8 PSUM banks) so real matmuls start as soon as attn chunk 0
    lands (~8.6 us) instead of waiting for the full 2 MB attn transfer.

  All matmul operands are fp16: full PE rate (1 row/cycle), ~5e-4
  element precision, half the HBM traffic of fp32.  PSUM accumulation
  is fp32; outputs are evicted as fp16 (host upcasts) to halve the
  output DMA.  x is host-pre-transposed (xT layout [1024, rows]) so the
  contraction dim lands on SBUF partitions with clean contiguous DMA.
  GELU (erf-exact) is fused into the PSUM->SBUF eviction on ScalarE.
"""

import numpy as np
from contextlib import ExitStack

import concourse.bass as bass  # noqa: F401  (engine registration side effects)
import concourse.mybir as mybir
import concourse.tile as tile
from concourse import bacc
from concourse import bass_utils

P = 128
B, D = 64, 768
IN_F, HID_F, OUT_F = 1024, 4096, 1024
N_CORES = 8
ROWS = B * D                    # 49152
ROWS_PC = ROWS // N_CORES       # 6144
S_TILES = IN_F // P             # 8
K_CHUNKS = IN_F // P            # 8
RT = ROWS_PC // P               # 48
NB = 512                        # matmul moving free dim / PSUM bank
KPC = HID_F // N_CORES          # 512 contraction rows per core (NEFF A)
KC = KPC // P                   # 4 k-chunks per core (NEFF A)
PRO = 4                         # NEFF B prologue row-blocks (uses all 8 banks)
WARM_A = 30                     # HAM warmup matmuls (~3.2 us at 1.2 GHz)
WARM_B = 30                     # bridge to first DMA completion (~11 us)

F32 = mybir.dt.float32
F16 = mybir.dt.float16


def _warmup(nc, pool, ppool, psum_name, n):
    """Dummy matmuls to open the PE HAM clock gate during the DMA fill.

    The psum scratch reuses the main pool's tag (same name) so it cycles
    through the same 8 bank slots instead of claiming its own tag group.
    """
    wz = pool.tile([P, P], F16, name="warm")
    nc.vector.memset(wz, 0.0)
    wp = ppool.tile([P, NB], F32, name=psum_name)
    for _ in range(n):
        nc.tensor.matmul(wp[:, 0:P], wz, wz, start=True, stop=True)


def _trace_attn_kernel(tc, partial, fc1c, fc2tc):
    """partial[1024,1024] (fp16) = fc1c^T @ fc2tc for this core's K-slice.

    fc1c  [512, 1024] fp16 : fc1 rows for this core's K-slice (s columns)
    fc2tc [512, 1024] fp16 : fc2^T rows for the same K-slice (t columns)
    """
    nc = tc.nc
    fc1_r = fc1c.rearrange("(k p) s -> p k s", p=P)      # [128, 4, 1024]
    fc2_r = fc2tc.rearrange("(k p) t -> p k t", p=P)     # [128, 4, 1024]
    # partial is [th, p, sb, t]: each DRAM partition row is 8 KB
    # contiguous, so the output DMA uses 8 KB descriptors instead of the
    # 1 KB ones a [s, t] layout would force (4x fewer, ~4x faster drain).
    out_r = partial.rearrange("(th p) (sb t) -> th p sb t", p=P, sb=8)

    with ExitStack() as ctx:
        spool = ctx.enter_context(tc.tile_pool(name="spool", bufs=1))
        opool = ctx.enter_context(tc.tile_pool(name="opool", bufs=1))
        ppool = ctx.enter_context(tc.tile_pool(name="ppool", bufs=8, space="PSUM"))
        _warmup(nc, spool, ppool, 'ap', WARM_A)
        f1_sb = spool.tile([P, KC, IN_F], F16)
        f2_sb = spool.tile([P, KC, OUT_F], F16)
        # Paired per-chunk DMAs on both HWDGE engines so chunk k lands
        # well ahead of the matmuls that consume it (one per 1.7 us).
        for k in range(KC):
            nc.sync.dma_start(f1_sb[:, k, :], fc1_r[:, k, :])
            nc.scalar.dma_start(f2_sb[:, k, :], fc2_r[:, k, :])
        # Wave 0 (t-half 0): k outer / s inner so each freshly arrived
        # chunk pair feeds all 8 stationary loads before the next chunk
        # is needed.  Evictions on VectorE, output via Sync.
        psums = [ppool.tile([P, NB], F32, name="ap") for _ in range(8)]
        for k in range(KC):
            for sb in range(8):
                nc.tensor.matmul(
                    psums[sb],
                    f1_sb[:, k, sb * P:(sb + 1) * P],
                    f2_sb[:, k, 0:NB],
                    start=(k == 0),
                    stop=(k == KC - 1),
                )
        ot0 = opool.tile([P, 8, NB], F16, name="ot0")
        for sb in range(8):
            # Evictions round-robin Vector/Scalar (~0.68 us each, and
            # GpSimd has no PSUM port) so the out DMA isn't gated on a
            # single engine draining all eight banks.
            if sb % 2 == 0:
                nc.vector.tensor_copy(ot0[:, sb, :], psums[sb])
            else:
                nc.scalar.copy(ot0[:, sb, :], psums[sb])
        nc.sync.dma_start(out_r[0], ot0)
        # Wave 1 (t-half 1): all data resident -> s outer / k inner so
        # each s-block completes early and is evicted while the next
        # computes; output in two halves via Scalar (idle by now).
        ot1 = opool.tile([P, 8, NB], F16, name="ot1")
        for sb in range(8):
            p = ppool.tile([P, NB], F32, name="ap")
            for k in range(KC):
                nc.tensor.matmul(
                    p,
                    f1_sb[:, k, sb * P:(sb + 1) * P],
                    f2_sb[:, k, NB:OUT_F],
                    start=(k == 0),
                    stop=(k == KC - 1),
                )
            if sb % 2 == 0:
                nc.vector.tensor_copy(ot1[:, sb, :], p)
            else:
                nc.scalar.copy(ot1[:, sb, :], p)
        nc.sync.dma_start(out_r[1], ot1)


def _trace_mlp_kernel(tc, out, attn, xt):
    """out[6144,1024] (fp16) = gelu(xT^T @ attn) for this core's row shard."""
    nc = tc.nc
    gelu = mybir.ActivationFunctionType.Gelu
    attn_r = attn.rearrange("(k p) t -> p k t", p=P)    # [128, 8, 1024]

    with ExitStack() as ctx:
        consts = ctx.enter_context(tc.tile_pool(name="consts", bufs=1))
        xpool = ctx.enter_context(tc.tile_pool(name="xpool", bufs=8))
        opool = ctx.enter_context(tc.tile_pool(name="opool", bufs=3))
        mpool = ctx.enter_context(tc.tile_pool(name="main_psum", bufs=8, space="PSUM"))
        _warmup(nc, consts, mpool, 'mp', WARM_B)
        attn_sb = consts.tile([P, S_TILES, OUT_F], F16)

        # Interleaved per-chunk attn / x-strip issues on Sync, ordered so
        # attn chunk k lands just before the prologue's k-th sweep.
        xs_t = []

        def xs_dma(rt):
            # xt is host-pre-shuffled to [rt, p, k, r]: each strip is 128
            # partition rows x 2 KB contiguous (128 DMA descriptors, not
            # 1024 256-byte ones).
            xs = xpool.tile([P, K_CHUNKS, P], F16, name="xs")
            nc.sync.dma_start(xs, xt[rt * P:(rt + 1) * P, :])
            return xs

        # attn chunks stream per-k on Sync; the prologue's x strips go
        # out in parallel on Scalar (idle until the first activations).
        for k in range(K_CHUNKS):
            nc.sync.dma_start(attn_sb[:, k:k + 1, :], attn_r[:, k:k + 1, :])
        for rt in range(PRO):
            xs = xpool.tile([P, K_CHUNKS, P], F16, name="xs")
            nc.scalar.dma_start(xs, xt[rt * P:(rt + 1) * P, :])
            xs_t.append(xs)

        # Prologue: k-major over row-blocks 0..3 (8 PSUM banks) — matmuls
        # start on attn chunk 0 instead of the full attn transfer.
        pro_ps = []
        for rt in range(PRO):
            pro_ps.append((mpool.tile([P, NB], F32, name="mp"),
                           mpool.tile([P, NB], F32, name="mp")))
        for k in range(K_CHUNKS):
            for rt in range(PRO):
                nc.tensor.matmul(
                    pro_ps[rt][0], xs_t[rt][:, k, :], attn_sb[:, k, 0:NB],
                    start=(k == 0), stop=(k == K_CHUNKS - 1),
                )
                nc.tensor.matmul(
                    pro_ps[rt][1], xs_t[rt][:, k, :], attn_sb[:, k, NB:OUT_F],
                    start=(k == 0), stop=(k == K_CHUNKS - 1),
                )
        for rt in range(PRO):
            ot = opool.tile([P, OUT_F], F16, name="ot")
            nc.scalar.activation(ot[:, 0:NB], pro_ps[rt][0], gelu)
            nc.scalar.activation(ot[:, NB:OUT_F], pro_ps[rt][1], gelu)
            nc.scalar.dma_start(out[rt * P:(rt + 1) * P, :], ot)

        # Steady state: row-block major; x strips prefetched 8 deep via
        # the pool; GELU eviction + output DMA issue both on ScalarE.
        for rt in range(PRO, RT):
            xs = xs_dma(rt)
            pa = mpool.tile([P, NB], F32, name="mp")
            pb = mpool.tile([P, NB], F32, name="mp")
            for k in range(K_CHUNKS):
                nc.tensor.matmul(
                    pa, xs[:, k, :], attn_sb[:, k, 0:NB],
                    start=(k == 0), stop=(k == K_CHUNKS - 1),
                )
            for k in range(K_CHUNKS):
                nc.tensor.matmul(
                    pb, xs[:, k, :], attn_sb[:, k, NB:OUT_F],
                    start=(k == 0), stop=(k == K_CHUNKS - 1),
                )
            ot = opool.tile([P, OUT_F], F16, name="ot")
            nc.scalar.activation(ot[:, 0:NB], pa, gelu)
            nc.scalar.activation(ot[:, NB:OUT_F], pb, gelu)
            nc.scalar.dma_start(out[rt * P:(rt + 1) * P, :], ot)


_NC_CACHE = {}
LAST_RESULTS = None


def _build_attn():
    if "attn" in _NC_CACHE:
        return _NC_CACHE["attn"]
    nc = bacc.Bacc("TRN2", target_bir_lowering=False, debug=False,
                   num_devices=N_CORES)
    fc1c = nc.dram_tensor("fc1c", [KPC, IN_F], F16, kind="ExternalInput").ap()
    fc2tc = nc.dram_tensor("fc2tc", [KPC, OUT_F], F16, kind="ExternalInput").ap()
    partial = nc.dram_tensor("partial", [2 * P, 8 * NB], F16,
                             kind="ExternalOutput").ap()
    with tile.TileContext(nc) as tc:
        _trace_attn_kernel(tc, partial, fc1c, fc2tc)
    nc.compile()
    _NC_CACHE["attn"] = nc
    return nc


def _build_mlp():
    if "mlp" in _NC_CACHE:
        return _NC_CACHE["mlp"]
    nc = bacc.Bacc("TRN2", target_bir_lowering=False, debug=False,
                   num_devices=N_CORES)
    attn = nc.dram_tensor("attn", [IN_F, OUT_F], F16, kind="ExternalInput").ap()
    xt = nc.dram_tensor("xt", [ROWS_PC, IN_F], F16, kind="ExternalInput").ap()
    out = nc.dram_tensor("out", [ROWS_PC, OUT_F], F16, kind="ExternalOutput").ap()
    with tile.TileContext(nc) as tc:
        _trace_mlp_kernel(tc, out, attn, xt)
    nc.compile()
    _NC_CACHE["mlp"] = nc
    return nc


def _run(nc, in_maps, **kwargs):
    return bass_utils.run_bass_kernel_spmd(
        nc, in_maps, core_ids=list(range(N_CORES)), **kwargs
    )


def kernel(x, fc1_w, fc2_w, fc2_b, sparse_mask, **run_kwargs):
    global LAST_RESULTS
    nc_a = _build_attn()
    nc_b = _build_mlp()

    # --- host prep: fp16 K-slices of the weights (layout only) ---
    fc1_16 = np.asarray(fc1_w, np.float32).astype(np.float16)      # [4096, 1024]
    fc2t_16 = np.asarray(fc2_w, np.float32).T.astype(np.float16)   # [4096, 1024]

    in_maps_a = []
    for c in range(N_CORES):
        sl = slice(c * KPC, (c + 1) * KPC)
        in_maps_a.append({
            "fc1c": np.ascontiguousarray(fc1_16[sl]),
            "fc2tc": np.ascontiguousarray(fc2t_16[sl]),
        })

    res_a = _run(nc_a, in_maps_a, **run_kwargs)

    # --- host: sum K-partials, add bias, apply mask (elementwise glue) ---
    acc = np.zeros((2, P, 8, NB), np.float32)
    for c in range(N_CORES):
        acc += res_a.results[c]["partial"].reshape(2, P, 8, NB)
    # [th, p, sb, t] -> [sb*128+p, th*512+t]
    acc = acc.transpose(2, 1, 0, 3).reshape(IN_F, OUT_F)
    attn_full = ((acc + np.asarray(fc2_b, np.float32))
                 * np.asarray(sparse_mask, np.float32)).astype(np.float16)

    x_flat = np.asarray(x, np.float32).reshape(ROWS, IN_F)
    in_maps_b = []
    for c in range(N_CORES):
        # [rt, r, k, p] -> [rt, p, k, r]: strip rt*128+p holds the 8
        # stationary k-blocks for x row-block rt contiguously.
        xs_c = np.ascontiguousarray(
            x_flat[c * ROWS_PC:(c + 1) * ROWS_PC]
            .reshape(RT, P, K_CHUNKS, P)
            .transpose(0, 3, 2, 1)
            .reshape(ROWS_PC, IN_F)
            .astype(np.float16)
        )
        in_maps_b.append({"attn": attn_full, "xt": xs_c})

    res_b = _run(nc_b, in_maps_b, **run_kwargs)
    LAST_RESULTS = (res_a, res_b)
    out = np.concatenate(
        [res_b.results[c]["out"] for c in range(N_CORES)], axis=0
    ).astype(np.float32)
    return out.reshape(B, D, OUT_F)
